# revision 22
# baseline (speedup 1.0000x reference)
"""BandsVQAutoencoder forward on 8 Trainium2 NeuronCores (Bass/Tile).

Strategy: data-parallel over batch (2 batches -> 2000 frames per core,
padded to 2048). Feature-major on-chip layout (features on partitions,
frames on the free axis) so every MLP bias is per-partition and weights are
natural lhsT operands.

Precision: encoder and VQ-cross matmuls run as 3-term fp16 hi/lo splits
(z*c ~= zh*ch + zh*cl + zl*ch, each 1 PE cycle/row) which reproduces fp32
accuracy to ~1e-8 -- required because the VQ argmin top-2 gap can be as
small as 1e-7. Decoder runs single fp16 (only feeds bands_hat).

VQ: z is laid out as 3 blocks of 97 partition rows (3 groups x 32 dims +
one shared ones-row), so the cross matmul for each group contracts the
whole 97-row block against a block-diagonal codebook rhs whose 97th row
carries -0.5*||c_k||^2 (the argmax-equivalent bias). score = z.c - cn/2
lands in PSUM; DVE max8/max_index extract each group's argmax directly
from PSUM; GpSimd indirect-DMA gathers the winning codewords from the
flat fp32 codebook in DRAM.

The chunk loop is software-pipelined: chunk c+1's encoder matmuls are
emitted interleaved into chunk c's VQ phase so the PE queue never drains
behind the DVE argmax (keeps the HAM clock gate warm).
"""

import os
import numpy as np

import concourse.bass as bass
import concourse.bacc as bacc
import concourse.mybir as mybir
import concourse.tile as tile
from concourse.bass_utils import run_bass_kernel_spmd
from concourse.masks import make_identity

F32 = mybir.dt.float32
F16 = mybir.dt.float16
U32 = mybir.dt.uint32

NUM_BANDS = 64
LATENT = 256
HIDDEN = 512
G = 8
K = 1024
GD = 32
BETA = 0.25

N_CORES = 8
B, T = 16, 1000
NC_FRAMES = 2000       # real frames per core (2 batches)
NPAD = 2048            # padded frames per core
FCH = 512              # frame chunk (matmul moving dim)
NCHUNK = NPAD // FCH   # 4
FT = 128               # frame tile (partition dim for VQ)
NTILE = FCH // FT      # 4 tiles per chunk

ZBLK = 3               # z blocks of [97 rows: 3 groups x 32 dims + ones row]
ZR = 97
ZW = 384               # 3 blocks x 128 w2_aug columns

_CACHE = {}
LAST_EXEC_NS = None

if int(os.environ.get("KERNEL_LDWOPT", "0")):
    import concourse.bass_utils as _bu
    if not getattr(_bu, "_ldwopt_patched", False):
        _orig_run_command = _bu.run_command

        def _run_command_ldwopt(argv, **kw):
            argv = [a.replace("--enable-ldw-opt=false", "--enable-ldw-opt=true")
                    if isinstance(a, str) else a for a in argv]
            return _orig_run_command(argv, **kw)

        _bu.run_command = _run_command_ldwopt
        _bu._ldwopt_patched = True


def _split16(a):
    h = a.astype(np.float16)
    l = (a.astype(np.float32) - h.astype(np.float32)).astype(np.float16)
    return h, l


def _build():
    nc = bacc.Bacc("TRN2", target_bir_lowering=False, debug=False)

    d = {}
    def din(name, shape, dt):
        d[name] = nc.dram_tensor(name, shape, dt, kind="ExternalInput").ap()
    def dout(name, shape, dt):
        d[name] = nc.dram_tensor(name, shape, dt, kind="ExternalOutput").ap()

    din("xh", [65, NPAD], F16); din("xl", [65, NPAD], F16)
    din("w1h", [65, HIDDEN], F16); din("w1l", [65, HIDDEN], F16)
    din("w2h", [HIDDEN + 1, ZW], F16); din("w2l", [HIDDEN + 1, ZW], F16)
    din("cbh", [ZBLK * ZR, 3 * K], F16); din("cbl", [ZBLK * ZR, 3 * K], F16)
    din("cbflat", [G * K, GD], F32)
    din("dw1", [LATENT + 1, HIDDEN], F16)
    din("dw2", [HIDDEN + 1, NUM_BANDS], F16)

    dout("zh_out", [ZW, NPAD], F16); dout("zl_out", [ZW, NPAD], F16)
    dout("zq_out", [NPAD, LATENT], F32)
    dout("idx_out", [NPAD, G], U32)
    dout("bh_out", [NUM_BANDS, NPAD], F32)

    with tile.TileContext(nc) as tc:
        wp = tc.alloc_tile_pool(name="w", bufs=1)
        sp = tc.alloc_tile_pool(name="s", bufs=2)
        pp = tc.alloc_tile_pool(name="ps", bufs=2, space="PSUM")
        dpp = tc.alloc_tile_pool(name="psd", bufs=3, space="PSUM")

        # ---- persistent weights / constants ----
        w1h = wp.tile([65, HIDDEN], F16, name="w1h"); nc.sync.dma_start(w1h[:], d["w1h"][:])
        w1l = wp.tile([65, HIDDEN], F16, name="w1l"); nc.sync.dma_start(w1l[:], d["w1l"][:])
        w2h = [wp.tile([128, ZW], F16, tag=f"w2h{k}", name=f"w2h{k}") for k in range(4)]
        w2l = [wp.tile([128, ZW], F16, tag=f"w2l{k}", name=f"w2l{k}") for k in range(4)]
        for k in range(4):
            nc.sync.dma_start(w2h[k][:], d["w2h"][128 * k:128 * (k + 1), :])
            nc.sync.dma_start(w2l[k][:], d["w2l"][128 * k:128 * (k + 1), :])
        w2hb = wp.tile([1, ZW], F16, name="w2hb"); nc.sync.dma_start(w2hb[:], d["w2h"][512:513, :])
        w2lb = wp.tile([1, ZW], F16, name="w2lb"); nc.sync.dma_start(w2lb[:], d["w2l"][512:513, :])
        cbh = [wp.tile([ZR, 3 * K], F16, tag=f"cbh{i}", name=f"cbh{i}") for i in range(ZBLK)]
        cbl = [wp.tile([ZR, 3 * K], F16, tag=f"cbl{i}", name=f"cbl{i}") for i in range(ZBLK)]
        for i in range(ZBLK):
            nc.sync.dma_start(cbh[i][:], d["cbh"][ZR * i:ZR * (i + 1), :])
            nc.sync.dma_start(cbl[i][:], d["cbl"][ZR * i:ZR * (i + 1), :])
        dw1 = [wp.tile([128, HIDDEN], F16, tag=f"dw1{k}", name=f"dw1{k}") for k in range(2)]
        for k in range(2):
            nc.sync.dma_start(dw1[k][:], d["dw1"][128 * k:128 * (k + 1), :])
        dw1b = wp.tile([1, HIDDEN], F16, name="dw1b"); nc.sync.dma_start(dw1b[:], d["dw1"][256:257, :])
        dw2 = [wp.tile([128, NUM_BANDS], F16, tag=f"dw2{k}", name=f"dw2{k}") for k in range(4)]
        for k in range(4):
            nc.sync.dma_start(dw2[k][:], d["dw2"][128 * k:128 * (k + 1), :])
        dw2b = wp.tile([1, NUM_BANDS], F16, name="dw2b"); nc.sync.dma_start(dw2b[:], d["dw2"][512:513, :])

        ident = wp.tile([128, 128], F16, name="ident")
        make_identity(nc, ident[:])
        ones_row = wp.tile([1, FCH], F16, name="ones_row"); nc.vector.memset(ones_row[:], 1.0)

        # ---- software-pipelined chunk loop ----
        chunk_z = {}

        def enc_gen(c):
            cs = slice(FCH * c, FCH * (c + 1))
            xh = sp.tile([65, FCH], F16, tag="xh", name="xh")
            xl = sp.tile([65, FCH], F16, tag="xl", name="xl")
            nc.sync.dma_start(xh[:], d["xh"][:, cs])
            nc.sync.dma_start(xl[:], d["xl"][:, cs])

            hh = []
            hl = []
            for m in range(4):
                ps = pp.tile([128, FCH], F32, tag="mlp", name="psmlp")
                ms = slice(128 * m, 128 * (m + 1))
                nc.tensor.matmul(ps[:], w1h[:, ms], xh[:], start=True, stop=False)
                nc.tensor.matmul(ps[:], w1h[:, ms], xl[:], start=False, stop=False)
                nc.tensor.matmul(ps[:], w1l[:, ms], xh[:], start=False, stop=True)
                th = sp.tile([128, FCH], F16, tag=f"hh{m}", name=f"hh{m}")
                nc.scalar.activation(th[:], ps[:], mybir.ActivationFunctionType.Relu)
                tl = sp.tile([128, FCH], F16, tag=f"hl{m}", name=f"hl{m}")
                nc.vector.scalar_tensor_tensor(
                    out=tl[:], in0=ps[:], scalar=0.0, in1=th[:],
                    op0=mybir.AluOpType.max, op1=mybir.AluOpType.subtract)
                hh.append(th); hl.append(tl)
                yield

            zh = []
            zl = []
            for i in range(ZBLK):
                msl = slice(128 * i, 128 * (i + 1))
                ps = pp.tile([128, FCH], F32, tag="mlp", name="psmlp")
                first = True
                for k in range(4):
                    nc.tensor.matmul(ps[:], w2h[k][:, msl], hh[k][:],
                                     start=first, stop=False); first = False
                nc.tensor.matmul(ps[:], w2hb[:, msl], ones_row[:],
                                 start=False, stop=False)
                for k in range(4):
                    nc.tensor.matmul(ps[:], w2l[k][:, msl], hh[k][:],
                                     start=False, stop=False)
                nc.tensor.matmul(ps[:], w2lb[:, msl], ones_row[:],
                                 start=False, stop=False)
                for k in range(4):
                    nc.tensor.matmul(ps[:], w2h[k][:, msl], hl[k][:],
                                     start=False, stop=(k == 3))
                th = sp.tile([ZR, FCH], F16, tag=f"zh{i}", name=f"zh{i}")
                nc.scalar.activation(th[:], ps[:ZR, :],
                                     mybir.ActivationFunctionType.Copy)
                tl = sp.tile([ZR, FCH], F16, tag=f"zl{i}", name=f"zl{i}")
                nc.vector.tensor_tensor(out=tl[:], in0=ps[:ZR, :], in1=th[:],
                                        op=mybir.AluOpType.subtract)
                nc.scalar.dma_start(d["zh_out"][128 * i:128 * i + ZR, cs], th[:])
                nc.scalar.dma_start(d["zl_out"][128 * i:128 * i + ZR, cs], tl[:])
                zh.append(th); zl.append(tl)
                yield
            chunk_z[c] = (zh, zl)

        for _ in enc_gen(0):
            pass

        for c in range(NCHUNK):
            cs = slice(FCH * c, FCH * (c + 1))
            zh, zl = chunk_z[c]
            nxt = enc_gen(c + 1) if c + 1 < NCHUNK else iter(())

            for t in range(NTILE):
                ts = slice(FT * t, FT * (t + 1))
                mxall = sp.tile([128, 8 * G], F32, tag="mxall", name="mxall",
                                bufs=3)
                idx8 = sp.tile([128, 8 * G], U32, tag="idx8", name="idx8",
                               bufs=3)
                for g in range(G):
                    i, j = divmod(g, 3)
                    dps = dpp.tile([128, K], F32, tag="dps", name="dps")
                    for kc in range(2):
                        ks = slice(512 * kc, 512 * (kc + 1))
                        cks = slice(K * j + 512 * kc, K * j + 512 * (kc + 1))
                        nc.tensor.matmul(dps[:, ks], zh[i][:, ts], cbh[i][:, cks],
                                         start=True, stop=False)
                        nc.tensor.matmul(dps[:, ks], zh[i][:, ts], cbl[i][:, cks],
                                         start=False, stop=False)
                    for kc in range(2):
                        ks = slice(512 * kc, 512 * (kc + 1))
                        cks = slice(K * j + 512 * kc, K * j + 512 * (kc + 1))
                        nc.tensor.matmul(dps[:, ks], zl[i][:, ts], cbh[i][:, cks],
                                         start=False, stop=(kc == 1))
                    # per-group argmax straight from PSUM (top-8 + first index)
                    e8 = slice(8 * g, 8 * (g + 1))
                    nc.vector.max(out=mxall[:, e8], in_=dps[:])
                    nc.vector.max_index(out=idx8[:, e8], in_max=mxall[:, e8],
                                        in_values=dps[:])
                    if g % 4 == 3:
                        next(nxt, None)
                idx = sp.tile([128, G], U32, tag="idx", name="idx", bufs=3)
                nc.vector.tensor_copy(
                    idx[:], idx8[:].rearrange("p (g e) -> p g e", g=G)[:, :, 0])
                nc.scalar.dma_start(d["idx_out"][FCH * c + FT * t:
                                                 FCH * c + FT * (t + 1), :], idx[:])

                zq = sp.tile([128, LATENT], F32, tag="zq", name="zq", bufs=3)
                for g in range(G):
                    nc.gpsimd.indirect_dma_start(
                        out=zq[:, GD * g:GD * (g + 1)], out_offset=None,
                        in_=d["cbflat"][:],
                        in_offset=bass.IndirectOffsetOnAxis(
                            ap=idx8[:, 8 * g:8 * g + 1], axis=0),
                        element_offset=g * K * GD,
                        bounds_check=K - 1, oob_is_err=False)
                nc.scalar.dma_start(d["zq_out"][FCH * c + FT * t:
                                                FCH * c + FT * (t + 1), :], zq[:])

                zq16 = sp.tile([128, LATENT], F16, tag="zq16", name="zq16",
                               bufs=3)
                nc.scalar.activation(zq16[:], zq[:], mybir.ActivationFunctionType.Copy)
                tp = pp.tile([128, LATENT], F16, tag="mlp", name="tp")
                nc.tensor.transpose(tp[:, 0:128], zq16[:, 0:128], ident[:])
                nc.tensor.transpose(tp[:, 128:256], zq16[:, 128:256], ident[:])
                if t == 0:
                    zqT = [sp.tile([128, FCH], F16, tag=f"zqT{k}", name=f"zqT{k}") for k in range(2)]
                nc.scalar.activation(zqT[0][:, ts], tp[:, 0:128],
                                     mybir.ActivationFunctionType.Copy)
                nc.scalar.activation(zqT[1][:, ts], tp[:, 128:256],
                                     mybir.ActivationFunctionType.Copy)

            for _ in nxt:
                pass

            # decoder layer 1
            d1 = []
            for m in range(4):
                ps = pp.tile([128, FCH], F32, tag="mlp", name="psmlp")
                ms = slice(128 * m, 128 * (m + 1))
                nc.tensor.matmul(ps[:], dw1[0][:, ms], zqT[0][:], start=True, stop=False)
                nc.tensor.matmul(ps[:], dw1[1][:, ms], zqT[1][:], start=False, stop=False)
                nc.tensor.matmul(ps[:], dw1b[:, ms], ones_row[:], start=False, stop=True)
                td = sp.tile([128, FCH], F16, tag=f"d1{m}", name=f"d1{m}")
                nc.scalar.activation(td[:], ps[:], mybir.ActivationFunctionType.Relu)
                d1.append(td)

            # decoder layer 2
            ps = pp.tile([64, FCH], F32, tag="mlp", name="psmlp2")
            for k in range(4):
                nc.tensor.matmul(ps[:], dw2[k][:], d1[k][:], start=(k == 0), stop=False)
            nc.tensor.matmul(ps[:], dw2b[:], ones_row[:], start=False, stop=True)
            bh = sp.tile([64, FCH], F32, tag="bh", name="bhT")
            nc.scalar.activation(bh[:], ps[:], mybir.ActivationFunctionType.Copy)
            nc.scalar.dma_start(d["bh_out"][:, cs], bh[:])

        for p in (dpp, pp, sp, wp):
            p.release()

    nc.compile()
    return nc


def _prep(inputs):
    w1 = np.asarray(inputs["enc_w1"], np.float32)
    b1 = np.asarray(inputs["enc_b1"], np.float32)
    w2 = np.asarray(inputs["enc_w2"], np.float32)
    b2 = np.asarray(inputs["enc_b2"], np.float32)
    cb = np.asarray(inputs["codebooks"], np.float32)
    dw1 = np.asarray(inputs["dec_w1"], np.float32)
    db1 = np.asarray(inputs["dec_b1"], np.float32)
    dw2 = np.asarray(inputs["dec_w2"], np.float32)
    db2 = np.asarray(inputs["dec_b2"], np.float32)

    w1a = np.vstack([w1, b1[None]])                       # (65, 512)
    w1h, w1l = _split16(w1a)

    # w2_aug: 3 blocks of 128 output columns; block i column q:
    #   q in [0,96): z dim of group 3i + q//32, q == 96: ones row
    # cb_bd: block-diagonal codebook per block, row 96 = -0.5*||c_k||^2
    w2a = np.zeros((HIDDEN + 1, ZW), np.float32)
    cba = np.zeros((ZBLK * ZR, 3 * K), np.float32)
    for g in range(G):
        i, j = divmod(g, 3)
        w2a[:HIDDEN, 128 * i + GD * j:128 * i + GD * (j + 1)] = \
            w2[:, GD * g:GD * (g + 1)]
        w2a[HIDDEN, 128 * i + GD * j:128 * i + GD * (j + 1)] = \
            b2[GD * g:GD * (g + 1)]
        cba[ZR * i + GD * j:ZR * i + GD * (j + 1), K * j:K * (j + 1)] = cb[g].T
        cba[ZR * i + 96, K * j:K * (j + 1)] = -0.5 * (cb[g] ** 2).sum(-1)
    for i in range(ZBLK):
        w2a[HIDDEN, 128 * i + 96] = 1.0
    w2h, w2l = _split16(w2a)
    cbh, cbl = _split16(cba)

    dw1a = np.vstack([dw1, db1[None]]).astype(np.float16)  # (257, 512)
    dw2a = np.vstack([dw2, db2[None]]).astype(np.float16)  # (513, 64)

    shared = dict(w1h=w1h, w1l=w1l, w2h=w2h, w2l=w2l, cbh=cbh, cbl=cbl,
                  cbflat=np.ascontiguousarray(cb.reshape(G * K, GD)),
                  dw1=dw1a, dw2=dw2a)

    bands = np.asarray(inputs["bands"], np.float32)
    in_maps = []
    for cix in range(N_CORES):
        xc = bands[2 * cix:2 * cix + 2].reshape(NC_FRAMES, NUM_BANDS)
        xT = np.zeros((65, NPAD), np.float32)
        xT[:NUM_BANDS, :NC_FRAMES] = xc.T
        xT[NUM_BANDS, :] = 1.0
        xh, xl = _split16(xT)
        in_maps.append(dict(shared, xh=xh, xl=xl))
    return in_maps


def kernel(**inputs):
    global LAST_EXEC_NS
    if "nc" not in _CACHE:
        _CACHE["nc"] = _build()
    nc = _CACHE["nc"]

    in_maps = _prep(inputs)
    trace = bool(int(os.environ.get("KERNEL_TRACE", "0")))
    kw = {}
    pdir = os.environ.get("KERNEL_PROF_DIR")
    if pdir:
        os.makedirs(pdir, exist_ok=True)
        kw["tmpdir"] = pdir
    res = run_bass_kernel_spmd(nc, in_maps, core_ids=list(range(N_CORES)),
                               trace=trace, **kw)
    LAST_EXEC_NS = res.exec_time_ns

    z_e = np.empty((N_CORES, NC_FRAMES, LATENT), np.float32)
    z_q = np.empty((N_CORES, NC_FRAMES, LATENT), np.float32)
    idx = np.empty((N_CORES, NC_FRAMES, G), np.int32)
    bh = np.empty((N_CORES, NC_FRAMES, NUM_BANDS), np.float32)
    for cix in range(N_CORES):
        r = res.results[cix]
        zT = r["zh_out"].astype(np.float32) + r["zl_out"].astype(np.float32)
        for g in range(G):
            i, j = divmod(g, 3)
            base = 128 * i + GD * j
            z_e[cix, :, GD * g:GD * (g + 1)] = zT[base:base + GD, :NC_FRAMES].T
        z_q[cix] = r["zq_out"][:NC_FRAMES]
        idx[cix] = r["idx_out"][:NC_FRAMES].astype(np.int32)
        bh[cix] = r["bh_out"][:, :NC_FRAMES].T

    z_e = z_e.reshape(B, T, LATENT)
    z_q = z_q.reshape(B, T, LATENT)
    idx = idx.reshape(B, T, G)
    bands_hat = bh.reshape(B, T, NUM_BANDS)

    z_q_st = z_e + (z_q - z_e)

    dif = (z_q - z_e).reshape(B * T, G, GD)
    m = np.mean(dif * dif, axis=(0, 2), dtype=np.float32)
    cl = m.sum(dtype=np.float32)
    vq_loss = np.float32(BETA) * (cl + cl)

    return (bands_hat, z_e, z_q_st, idx, np.float32(vq_loss))


# revision 25
# speedup vs baseline: 1.0017x; 1.0017x over previous
"""BandsVQAutoencoder forward on 8 Trainium2 NeuronCores (Bass/Tile).

Strategy: data-parallel over batch (2 batches -> 2000 frames per core,
padded to 2048). Feature-major on-chip layout (features on partitions,
frames on the free axis) so every MLP bias is per-partition and weights are
natural lhsT operands.

Precision: encoder and VQ-cross matmuls run as 3-term fp16 hi/lo splits
(z*c ~= zh*ch + zh*cl + zl*ch, each 1 PE cycle/row) which reproduces fp32
accuracy to ~1e-8 -- required because the VQ argmin top-2 gap can be as
small as 1e-7. Decoder runs single fp16 (only feeds bands_hat).

VQ: z is laid out as 3 blocks of 97 partition rows (3 groups x 32 dims +
one shared ones-row), so the cross matmul for each group contracts the
whole 97-row block against a block-diagonal codebook rhs whose 97th row
carries -0.5*||c_k||^2 (the argmax-equivalent bias). score = z.c - cn/2
lands in PSUM; DVE max8/max_index extract each group's argmax directly
from PSUM; GpSimd indirect-DMA gathers the winning codewords from the
flat fp32 codebook in DRAM.

The chunk loop is software-pipelined: chunk c+1's encoder matmuls are
emitted interleaved into chunk c's VQ phase so the PE queue never drains
behind the DVE argmax (keeps the HAM clock gate warm).
"""

import os
import numpy as np

import concourse.bass as bass
import concourse.bacc as bacc
import concourse.mybir as mybir
import concourse.tile as tile
from concourse.bass_utils import run_bass_kernel_spmd
from concourse.masks import make_identity

F32 = mybir.dt.float32
F16 = mybir.dt.float16
U32 = mybir.dt.uint32

NUM_BANDS = 64
LATENT = 256
HIDDEN = 512
G = 8
K = 1024
GD = 32
BETA = 0.25

N_CORES = 8
B, T = 16, 1000
NC_FRAMES = 2000       # real frames per core (2 batches)
NPAD = 2048            # padded frames per core
FCH = 512              # frame chunk (matmul moving dim)
NCHUNK = NPAD // FCH   # 4
FT = 128               # frame tile (partition dim for VQ)
NTILE = FCH // FT      # 4 tiles per chunk

ZBLK = 3               # z blocks of [97 rows: 3 groups x 32 dims + ones row]
ZR = 97
ZW = 384               # 3 blocks x 128 w2_aug columns

_CACHE = {}
LAST_EXEC_NS = None

if int(os.environ.get("KERNEL_LDWOPT", "0")):
    import concourse.bass_utils as _bu
    if not getattr(_bu, "_ldwopt_patched", False):
        _orig_run_command = _bu.run_command

        def _run_command_ldwopt(argv, **kw):
            argv = [a.replace("--enable-ldw-opt=false", "--enable-ldw-opt=true")
                    if isinstance(a, str) else a for a in argv]
            return _orig_run_command(argv, **kw)

        _bu.run_command = _run_command_ldwopt
        _bu._ldwopt_patched = True


def _split16(a):
    h = a.astype(np.float16)
    l = (a.astype(np.float32) - h.astype(np.float32)).astype(np.float16)
    return h, l


def _build(b2_zero, db1_zero, db2_zero):
    nc = bacc.Bacc("TRN2", target_bir_lowering=False, debug=False)

    d = {}
    def din(name, shape, dt):
        d[name] = nc.dram_tensor(name, shape, dt, kind="ExternalInput").ap()
    def dout(name, shape, dt):
        d[name] = nc.dram_tensor(name, shape, dt, kind="ExternalOutput").ap()

    din("xh", [65, NPAD], F16); din("xl", [65, NPAD], F16)
    din("w1h", [65, HIDDEN], F16); din("w1l", [65, HIDDEN], F16)
    din("w2h", [HIDDEN + 1, ZW], F16); din("w2l", [HIDDEN + 1, ZW], F16)
    din("cbh", [ZBLK * ZR, 3 * K], F16); din("cbl", [ZBLK * ZR, 3 * K], F16)
    din("cbflat", [G * K, GD], F32)
    din("dw1", [LATENT + 1, HIDDEN], F16)
    din("dw2", [HIDDEN + 1, NUM_BANDS], F16)

    dout("zh_out", [ZW, NPAD], F16); dout("zl_out", [ZW, NPAD], F16)
    dout("zq_out", [NPAD, LATENT], F32)
    dout("idx_out", [NPAD, G], U32)
    dout("bh_out", [NUM_BANDS, NPAD], F32)

    with tile.TileContext(nc) as tc:
        wp = tc.alloc_tile_pool(name="w", bufs=1)
        sp = tc.alloc_tile_pool(name="s", bufs=2)
        pp = tc.alloc_tile_pool(name="ps", bufs=2, space="PSUM")
        dpp = tc.alloc_tile_pool(name="psd", bufs=3, space="PSUM")

        # ---- persistent weights / constants ----
        w1h = wp.tile([65, HIDDEN], F16, name="w1h"); nc.sync.dma_start(w1h[:], d["w1h"][:])
        w1l = wp.tile([65, HIDDEN], F16, name="w1l"); nc.sync.dma_start(w1l[:], d["w1l"][:])
        w2h = [wp.tile([128, ZW], F16, tag=f"w2h{k}", name=f"w2h{k}") for k in range(4)]
        w2l = [wp.tile([128, ZW], F16, tag=f"w2l{k}", name=f"w2l{k}") for k in range(4)]
        for k in range(4):
            nc.sync.dma_start(w2h[k][:], d["w2h"][128 * k:128 * (k + 1), :])
            nc.sync.dma_start(w2l[k][:], d["w2l"][128 * k:128 * (k + 1), :])
        w2hb = wp.tile([1, ZW], F16, name="w2hb"); nc.sync.dma_start(w2hb[:], d["w2h"][512:513, :])
        w2lb = wp.tile([1, ZW], F16, name="w2lb"); nc.sync.dma_start(w2lb[:], d["w2l"][512:513, :])
        cbh = [wp.tile([ZR, 3 * K], F16, tag=f"cbh{i}", name=f"cbh{i}") for i in range(ZBLK)]
        cbl = [wp.tile([ZR, 3 * K], F16, tag=f"cbl{i}", name=f"cbl{i}") for i in range(ZBLK)]
        for i in range(ZBLK):
            nc.sync.dma_start(cbh[i][:], d["cbh"][ZR * i:ZR * (i + 1), :])
            nc.sync.dma_start(cbl[i][:], d["cbl"][ZR * i:ZR * (i + 1), :])
        dw1 = [wp.tile([128, HIDDEN], F16, tag=f"dw1{k}", name=f"dw1{k}") for k in range(2)]
        for k in range(2):
            nc.sync.dma_start(dw1[k][:], d["dw1"][128 * k:128 * (k + 1), :])
        dw1b = wp.tile([1, HIDDEN], F16, name="dw1b"); nc.sync.dma_start(dw1b[:], d["dw1"][256:257, :])
        dw2 = [wp.tile([128, NUM_BANDS], F16, tag=f"dw2{k}", name=f"dw2{k}") for k in range(4)]
        for k in range(4):
            nc.sync.dma_start(dw2[k][:], d["dw2"][128 * k:128 * (k + 1), :])
        dw2b = wp.tile([1, NUM_BANDS], F16, name="dw2b"); nc.sync.dma_start(dw2b[:], d["dw2"][512:513, :])

        ident = wp.tile([128, 128], F16, name="ident")
        make_identity(nc, ident[:])
        ones_row = wp.tile([1, FCH], F16, name="ones_row"); nc.vector.memset(ones_row[:], 1.0)

        # ---- software-pipelined chunk loop ----
        chunk_z = {}

        def enc_gen(c):
            cs = slice(FCH * c, FCH * (c + 1))
            xh = sp.tile([65, FCH], F16, tag="xh", name="xh")
            xl = sp.tile([65, FCH], F16, tag="xl", name="xl")
            nc.sync.dma_start(xh[:], d["xh"][:, cs])
            nc.sync.dma_start(xl[:], d["xl"][:, cs])

            hh = []
            hl = []
            for m in range(4):
                ps = pp.tile([128, FCH], F32, tag="mlp", name="psmlp")
                ms = slice(128 * m, 128 * (m + 1))
                nc.tensor.matmul(ps[:], w1h[:, ms], xh[:], start=True, stop=False)
                nc.tensor.matmul(ps[:], w1h[:, ms], xl[:], start=False, stop=False)
                nc.tensor.matmul(ps[:], w1l[:, ms], xh[:], start=False, stop=True)
                th = sp.tile([128, FCH], F16, tag=f"hh{m}", name=f"hh{m}")
                nc.scalar.activation(th[:], ps[:], mybir.ActivationFunctionType.Relu)
                tl = sp.tile([128, FCH], F16, tag=f"hl{m}", name=f"hl{m}")
                nc.vector.scalar_tensor_tensor(
                    out=tl[:], in0=ps[:], scalar=0.0, in1=th[:],
                    op0=mybir.AluOpType.max, op1=mybir.AluOpType.subtract)
                hh.append(th); hl.append(tl)
                yield

            zh = []
            zl = []
            for i in range(ZBLK):
                msl = slice(128 * i, 128 * (i + 1))
                ps = pp.tile([128, FCH], F32, tag="mlp", name="psmlp")
                first = True
                for k in range(4):
                    nc.tensor.matmul(ps[:], w2h[k][:, msl], hh[k][:],
                                     start=first, stop=False); first = False
                if not b2_zero:
                    nc.tensor.matmul(ps[:], w2hb[:, msl], ones_row[:],
                                     start=False, stop=False)
                for k in range(4):
                    nc.tensor.matmul(ps[:], w2l[k][:, msl], hh[k][:],
                                     start=False, stop=False)
                if not b2_zero:
                    nc.tensor.matmul(ps[:], w2lb[:, msl], ones_row[:],
                                     start=False, stop=False)
                for k in range(4):
                    nc.tensor.matmul(ps[:], w2h[k][:, msl], hl[k][:],
                                     start=False, stop=(k == 3))
                th = sp.tile([ZR, FCH], F16, tag=f"zh{i}", name=f"zh{i}")
                nc.scalar.activation(th[:], ps[:ZR, :],
                                     mybir.ActivationFunctionType.Copy)
                tl = sp.tile([ZR, FCH], F16, tag=f"zl{i}", name=f"zl{i}")
                nc.vector.tensor_tensor(out=tl[:], in0=ps[:ZR, :], in1=th[:],
                                        op=mybir.AluOpType.subtract)
                if b2_zero:
                    # the ones row (feeds the -0.5*||c||^2 bias term) normally
                    # rides the bias matmuls; set it directly instead
                    nc.vector.memset(th[96:97, :], 1.0)
                    nc.vector.memset(tl[96:97, :], 0.0)
                nc.scalar.dma_start(d["zh_out"][128 * i:128 * i + ZR, cs], th[:])
                nc.scalar.dma_start(d["zl_out"][128 * i:128 * i + ZR, cs], tl[:])
                zh.append(th); zl.append(tl)
                yield
            chunk_z[c] = (zh, zl)

        def dec_gen(c, zqT):
            cs = slice(FCH * c, FCH * (c + 1))
            d1 = []
            for m in range(4):
                ps = pp.tile([128, FCH], F32, tag="mlp", name="psmlp")
                ms = slice(128 * m, 128 * (m + 1))
                nc.tensor.matmul(ps[:], dw1[0][:, ms], zqT[0][:], start=True,
                                 stop=False)
                nc.tensor.matmul(ps[:], dw1[1][:, ms], zqT[1][:],
                                 start=False, stop=db1_zero)
                if not db1_zero:
                    nc.tensor.matmul(ps[:], dw1b[:, ms], ones_row[:],
                                     start=False, stop=True)
                td = sp.tile([128, FCH], F16, tag=f"d1{m}", name=f"d1{m}")
                nc.scalar.activation(td[:], ps[:], mybir.ActivationFunctionType.Relu)
                d1.append(td)
                yield
            ps = pp.tile([64, FCH], F32, tag="mlp", name="psmlp2")
            for k in range(4):
                nc.tensor.matmul(ps[:], dw2[k][:], d1[k][:], start=(k == 0),
                                 stop=(db2_zero and k == 3))
            if not db2_zero:
                nc.tensor.matmul(ps[:], dw2b[:], ones_row[:], start=False, stop=True)
            bh = sp.tile([64, FCH], F32, tag="bh", name="bhT")
            nc.scalar.activation(bh[:], ps[:], mybir.ActivationFunctionType.Copy)
            nc.scalar.dma_start(d["bh_out"][:, cs], bh[:])
            yield

        import itertools

        for _ in enc_gen(0):
            pass

        prev_dec = iter(())
        for c in range(NCHUNK):
            cs = slice(FCH * c, FCH * (c + 1))
            zh, zl = chunk_z[c]
            nxt = itertools.chain(
                prev_dec, enc_gen(c + 1) if c + 1 < NCHUNK else iter(()))

            for t in range(NTILE):
                ts = slice(FT * t, FT * (t + 1))
                mxall = sp.tile([128, 8 * G], F32, tag="mxall", name="mxall",
                                bufs=3)
                idx8 = sp.tile([128, 8 * G], U32, tag="idx8", name="idx8",
                               bufs=3)
                for g in range(G):
                    i, j = divmod(g, 3)
                    dps = dpp.tile([128, K], F32, tag="dps", name="dps")
                    for kc in range(2):
                        ks = slice(512 * kc, 512 * (kc + 1))
                        cks = slice(K * j + 512 * kc, K * j + 512 * (kc + 1))
                        nc.tensor.matmul(dps[:, ks], zh[i][:, ts], cbh[i][:, cks],
                                         start=True, stop=False)
                        nc.tensor.matmul(dps[:, ks], zh[i][:, ts], cbl[i][:, cks],
                                         start=False, stop=False)
                    for kc in range(2):
                        ks = slice(512 * kc, 512 * (kc + 1))
                        cks = slice(K * j + 512 * kc, K * j + 512 * (kc + 1))
                        nc.tensor.matmul(dps[:, ks], zl[i][:, ts], cbh[i][:, cks],
                                         start=False, stop=(kc == 1))
                    # per-group argmax straight from PSUM (top-8 + first index)
                    e8 = slice(8 * g, 8 * (g + 1))
                    nc.vector.max(out=mxall[:, e8], in_=dps[:])
                    nc.vector.max_index(out=idx8[:, e8], in_max=mxall[:, e8],
                                        in_values=dps[:])
                    if g % 3 == 2:
                        next(nxt, None)
                idx = sp.tile([128, G], U32, tag="idx", name="idx", bufs=3)
                nc.vector.tensor_copy(
                    idx[:], idx8[:].rearrange("p (g e) -> p g e", g=G)[:, :, 0])
                nc.scalar.dma_start(d["idx_out"][FCH * c + FT * t:
                                                 FCH * c + FT * (t + 1), :], idx[:])

                zq = sp.tile([128, LATENT], F32, tag="zq", name="zq", bufs=3)
                for g in range(G):
                    nc.gpsimd.indirect_dma_start(
                        out=zq[:, GD * g:GD * (g + 1)], out_offset=None,
                        in_=d["cbflat"][:],
                        in_offset=bass.IndirectOffsetOnAxis(
                            ap=idx8[:, 8 * g:8 * g + 1], axis=0),
                        element_offset=g * K * GD,
                        bounds_check=K - 1, oob_is_err=False)
                nc.scalar.dma_start(d["zq_out"][FCH * c + FT * t:
                                                FCH * c + FT * (t + 1), :], zq[:])

                zq16 = sp.tile([128, LATENT], F16, tag="zq16", name="zq16",
                               bufs=3)
                nc.scalar.activation(zq16[:], zq[:], mybir.ActivationFunctionType.Copy)
                tp = pp.tile([128, LATENT], F16, tag="mlp", name="tp")
                nc.tensor.transpose(tp[:, 0:128], zq16[:, 0:128], ident[:])
                nc.tensor.transpose(tp[:, 128:256], zq16[:, 128:256], ident[:])
                if t == 0:
                    zqT = [sp.tile([128, FCH], F16, tag=f"zqT{k}", name=f"zqT{k}") for k in range(2)]
                nc.scalar.activation(zqT[0][:, ts], tp[:, 0:128],
                                     mybir.ActivationFunctionType.Copy)
                nc.scalar.activation(zqT[1][:, ts], tp[:, 128:256],
                                     mybir.ActivationFunctionType.Copy)

            for _ in nxt:
                pass
            prev_dec = dec_gen(c, zqT)

        for _ in prev_dec:
            pass

        for p in (dpp, pp, sp, wp):
            p.release()

    nc.compile()
    return nc


def _prep(inputs):
    w1 = np.asarray(inputs["enc_w1"], np.float32)
    b1 = np.asarray(inputs["enc_b1"], np.float32)
    w2 = np.asarray(inputs["enc_w2"], np.float32)
    b2 = np.asarray(inputs["enc_b2"], np.float32)
    cb = np.asarray(inputs["codebooks"], np.float32)
    dw1 = np.asarray(inputs["dec_w1"], np.float32)
    db1 = np.asarray(inputs["dec_b1"], np.float32)
    dw2 = np.asarray(inputs["dec_w2"], np.float32)
    db2 = np.asarray(inputs["dec_b2"], np.float32)

    w1a = np.vstack([w1, b1[None]])                       # (65, 512)
    w1h, w1l = _split16(w1a)

    # w2_aug: 3 blocks of 128 output columns; block i column q:
    #   q in [0,96): z dim of group 3i + q//32, q == 96: ones row
    # cb_bd: block-diagonal codebook per block, row 96 = -0.5*||c_k||^2
    w2a = np.zeros((HIDDEN + 1, ZW), np.float32)
    cba = np.zeros((ZBLK * ZR, 3 * K), np.float32)
    for g in range(G):
        i, j = divmod(g, 3)
        w2a[:HIDDEN, 128 * i + GD * j:128 * i + GD * (j + 1)] = \
            w2[:, GD * g:GD * (g + 1)]
        w2a[HIDDEN, 128 * i + GD * j:128 * i + GD * (j + 1)] = \
            b2[GD * g:GD * (g + 1)]
        cba[ZR * i + GD * j:ZR * i + GD * (j + 1), K * j:K * (j + 1)] = cb[g].T
        cba[ZR * i + 96, K * j:K * (j + 1)] = -0.5 * (cb[g] ** 2).sum(-1)
    for i in range(ZBLK):
        w2a[HIDDEN, 128 * i + 96] = 1.0
    w2h, w2l = _split16(w2a)
    cbh, cbl = _split16(cba)

    dw1a = np.vstack([dw1, db1[None]]).astype(np.float16)  # (257, 512)
    dw2a = np.vstack([dw2, db2[None]]).astype(np.float16)  # (513, 64)

    shared = dict(w1h=w1h, w1l=w1l, w2h=w2h, w2l=w2l, cbh=cbh, cbl=cbl,
                  cbflat=np.ascontiguousarray(cb.reshape(G * K, GD)),
                  dw1=dw1a, dw2=dw2a)

    bands = np.asarray(inputs["bands"], np.float32)
    in_maps = []
    for cix in range(N_CORES):
        xc = bands[2 * cix:2 * cix + 2].reshape(NC_FRAMES, NUM_BANDS)
        xT = np.zeros((65, NPAD), np.float32)
        xT[:NUM_BANDS, :NC_FRAMES] = xc.T
        xT[NUM_BANDS, :] = 1.0
        xh, xl = _split16(xT)
        in_maps.append(dict(shared, xh=xh, xl=xl))
    return in_maps


def kernel(**inputs):
    global LAST_EXEC_NS
    flags = (not np.any(np.asarray(inputs["enc_b2"])),
             not np.any(np.asarray(inputs["dec_b1"])),
             not np.any(np.asarray(inputs["dec_b2"])))
    if flags not in _CACHE:
        _CACHE[flags] = _build(*flags)
    nc = _CACHE[flags]

    in_maps = _prep(inputs)
    trace = bool(int(os.environ.get("KERNEL_TRACE", "0")))
    kw = {}
    pdir = os.environ.get("KERNEL_PROF_DIR")
    if pdir:
        os.makedirs(pdir, exist_ok=True)
        kw["tmpdir"] = pdir
    res = run_bass_kernel_spmd(nc, in_maps, core_ids=list(range(N_CORES)),
                               trace=trace, **kw)
    LAST_EXEC_NS = res.exec_time_ns

    z_e = np.empty((N_CORES, NC_FRAMES, LATENT), np.float32)
    z_q = np.empty((N_CORES, NC_FRAMES, LATENT), np.float32)
    idx = np.empty((N_CORES, NC_FRAMES, G), np.int32)
    bh = np.empty((N_CORES, NC_FRAMES, NUM_BANDS), np.float32)
    for cix in range(N_CORES):
        r = res.results[cix]
        zT = r["zh_out"].astype(np.float32) + r["zl_out"].astype(np.float32)
        for g in range(G):
            i, j = divmod(g, 3)
            base = 128 * i + GD * j
            z_e[cix, :, GD * g:GD * (g + 1)] = zT[base:base + GD, :NC_FRAMES].T
        z_q[cix] = r["zq_out"][:NC_FRAMES]
        idx[cix] = r["idx_out"][:NC_FRAMES].astype(np.int32)
        bh[cix] = r["bh_out"][:, :NC_FRAMES].T

    z_e = z_e.reshape(B, T, LATENT)
    z_q = z_q.reshape(B, T, LATENT)
    idx = idx.reshape(B, T, G)
    bands_hat = bh.reshape(B, T, NUM_BANDS)

    z_q_st = z_e + (z_q - z_e)

    dif = (z_q - z_e).reshape(B * T, G, GD)
    m = np.mean(dif * dif, axis=(0, 2), dtype=np.float32)
    cl = m.sum(dtype=np.float32)
    vq_loss = np.float32(BETA) * (cl + cl)

    return (bands_hat, z_e, z_q_st, idx, np.float32(vq_loss))


# revision 26
# speedup vs baseline: 1.2717x; 1.2695x over previous
"""BandsVQAutoencoder forward on 8 Trainium2 NeuronCores (Bass/Tile).

Strategy: data-parallel over batch (2 batches -> 2000 frames per core,
padded to 2048). Feature-major on-chip layout (features on partitions,
frames on the free axis) so every MLP bias is per-partition and weights are
natural lhsT operands.

Precision: encoder and VQ-cross matmuls run as 3-term fp16 hi/lo splits
(z*c ~= zh*ch + zh*cl + zl*ch, each 1 PE cycle/row) which reproduces fp32
accuracy to ~1e-8 -- required because the VQ argmin top-2 gap can be as
small as 1e-7. Decoder runs single fp16 (only feeds bands_hat).

VQ: z is laid out as 3 blocks of 97 partition rows (3 groups x 32 dims +
one shared ones-row), so the cross matmul for each group contracts the
whole 97-row block against a block-diagonal codebook rhs whose 97th row
carries -0.5*||c_k||^2 (the argmax-equivalent bias). score = z.c - cn/2
lands in PSUM; DVE max8/max_index extract each group's argmax directly
from PSUM; GpSimd indirect-DMA gathers the winning codewords from the
flat fp32 codebook in DRAM.

The chunk loop is software-pipelined: chunk c+1's encoder matmuls are
emitted interleaved into chunk c's VQ phase so the PE queue never drains
behind the DVE argmax (keeps the HAM clock gate warm).
"""

import os
import numpy as np

import concourse.bass as bass
import concourse.bacc as bacc
import concourse.mybir as mybir
import concourse.tile as tile
from concourse.bass_utils import run_bass_kernel_spmd
from concourse.masks import make_identity

F32 = mybir.dt.float32
F16 = mybir.dt.float16
U32 = mybir.dt.uint32

NUM_BANDS = 64
LATENT = 256
HIDDEN = 512
G = 8
K = 1024
GD = 32
BETA = 0.25

N_CORES = 8
B, T = 16, 1000
NC_FRAMES = 2000       # real frames per core (2 batches)
NPAD = 2048            # padded frames per core
FCH = 512              # frame chunk (matmul moving dim)
NCHUNK = NPAD // FCH   # 4
FT = 128               # frame tile (partition dim for VQ)
NTILE = FCH // FT      # 4 tiles per chunk

ZBLK = 3               # z blocks of [97 rows: 3 groups x 32 dims + ones row]
ZR = 97
ZW = 384               # 3 blocks x 128 w2_aug columns

_CACHE = {}
LAST_EXEC_NS = None

if int(os.environ.get("KERNEL_LDWOPT", "0")):
    import concourse.bass_utils as _bu
    if not getattr(_bu, "_ldwopt_patched", False):
        _orig_run_command = _bu.run_command

        def _run_command_ldwopt(argv, **kw):
            argv = [a.replace("--enable-ldw-opt=false", "--enable-ldw-opt=true")
                    if isinstance(a, str) else a for a in argv]
            return _orig_run_command(argv, **kw)

        _bu.run_command = _run_command_ldwopt
        _bu._ldwopt_patched = True


def _split16(a):
    h = a.astype(np.float16)
    l = (a.astype(np.float32) - h.astype(np.float32)).astype(np.float16)
    return h, l


def _build(b2_zero, db1_zero, db2_zero):
    nc = bacc.Bacc("TRN2", target_bir_lowering=False, debug=False)

    d = {}
    def din(name, shape, dt):
        d[name] = nc.dram_tensor(name, shape, dt, kind="ExternalInput").ap()
    def dout(name, shape, dt):
        d[name] = nc.dram_tensor(name, shape, dt, kind="ExternalOutput").ap()

    din("xh", [65, NPAD], F16); din("xl", [65, NPAD], F16)
    din("w1h", [65, HIDDEN], F16); din("w1l", [65, HIDDEN], F16)
    din("w2h", [HIDDEN + 1, ZW], F16); din("w2l", [HIDDEN + 1, ZW], F16)
    din("cbhs", [G * GD, K], F16); din("cbls", [G * GD, K], F16)
    din("cbhb", [ZBLK, 3 * K], F16); din("cblb", [ZBLK, 3 * K], F16)
    din("cbflat", [G * K, GD], F32)
    din("dw1", [LATENT + 1, HIDDEN], F16)
    din("dw2", [HIDDEN + 1, NUM_BANDS], F16)

    dout("zh_out", [ZW, NPAD], F16); dout("zl_out", [ZW, NPAD], F16)
    dout("zq_out", [NPAD, LATENT], F32)
    dout("idx_out", [NPAD, G], U32)
    dout("bh_out", [NUM_BANDS, NPAD], F32)

    with tile.TileContext(nc) as tc:
        wp = tc.alloc_tile_pool(name="w", bufs=1)
        sp = tc.alloc_tile_pool(name="s", bufs=2)
        pp = tc.alloc_tile_pool(name="ps", bufs=2, space="PSUM")
        dpp = tc.alloc_tile_pool(name="psd", bufs=3, space="PSUM")

        # ---- persistent weights / constants ----
        w1h = wp.tile([65, HIDDEN], F16, name="w1h"); nc.sync.dma_start(w1h[:], d["w1h"][:])
        w1l = wp.tile([65, HIDDEN], F16, name="w1l"); nc.sync.dma_start(w1l[:], d["w1l"][:])
        w2h = [wp.tile([128, ZW], F16, tag=f"w2h{k}", name=f"w2h{k}") for k in range(4)]
        w2l = [wp.tile([128, ZW], F16, tag=f"w2l{k}", name=f"w2l{k}") for k in range(4)]
        for k in range(4):
            nc.sync.dma_start(w2h[k][:], d["w2h"][128 * k:128 * (k + 1), :])
            nc.sync.dma_start(w2l[k][:], d["w2l"][128 * k:128 * (k + 1), :])
        w2hb = wp.tile([1, ZW], F16, name="w2hb"); nc.sync.dma_start(w2hb[:], d["w2h"][512:513, :])
        w2lb = wp.tile([1, ZW], F16, name="w2lb"); nc.sync.dma_start(w2lb[:], d["w2l"][512:513, :])
        # block-diagonal codebook tiles: zero once, then DMA only the
        # nonzero 32-row strips and the bias row (8x fewer bytes)
        cbh = [wp.tile([ZR, 3 * K], F16, tag=f"cbh{i}", name=f"cbh{i}") for i in range(ZBLK)]
        cbl = [wp.tile([ZR, 3 * K], F16, tag=f"cbl{i}", name=f"cbl{i}") for i in range(ZBLK)]
        for i in range(ZBLK):
            nc.gpsimd.memset(cbh[i][:], 0.0)
            nc.gpsimd.memset(cbl[i][:], 0.0)
        for g in range(G):
            i, j = divmod(g, 3)
            nc.sync.dma_start(cbh[i][GD * j:GD * (j + 1), K * j:K * (j + 1)],
                              d["cbhs"][GD * g:GD * (g + 1), :])
            nc.scalar.dma_start(cbl[i][GD * j:GD * (j + 1), K * j:K * (j + 1)],
                                d["cbls"][GD * g:GD * (g + 1), :])
        for i in range(ZBLK):
            nc.sync.dma_start(cbh[i][96:97, :], d["cbhb"][i:i + 1, :])
            nc.scalar.dma_start(cbl[i][96:97, :], d["cblb"][i:i + 1, :])
        dw1 = [wp.tile([128, HIDDEN], F16, tag=f"dw1{k}", name=f"dw1{k}") for k in range(2)]
        for k in range(2):
            nc.scalar.dma_start(dw1[k][:], d["dw1"][128 * k:128 * (k + 1), :])
        dw1b = wp.tile([1, HIDDEN], F16, name="dw1b"); nc.scalar.dma_start(dw1b[:], d["dw1"][256:257, :])
        dw2 = [wp.tile([128, NUM_BANDS], F16, tag=f"dw2{k}", name=f"dw2{k}") for k in range(4)]
        for k in range(4):
            nc.scalar.dma_start(dw2[k][:], d["dw2"][128 * k:128 * (k + 1), :])
        dw2b = wp.tile([1, NUM_BANDS], F16, name="dw2b"); nc.scalar.dma_start(dw2b[:], d["dw2"][512:513, :])

        ident = wp.tile([128, 128], F16, name="ident")
        make_identity(nc, ident[:])
        ones_row = wp.tile([1, FCH], F16, name="ones_row"); nc.vector.memset(ones_row[:], 1.0)

        # ---- software-pipelined chunk loop ----
        chunk_z = {}

        def enc_gen(c):
            cs = slice(FCH * c, FCH * (c + 1))
            xh = sp.tile([65, FCH], F16, tag="xh", name="xh")
            xl = sp.tile([65, FCH], F16, tag="xl", name="xl")
            nc.sync.dma_start(xh[:], d["xh"][:, cs])
            nc.sync.dma_start(xl[:], d["xl"][:, cs])

            hh = []
            hl = []
            for m in range(4):
                ps = pp.tile([128, FCH], F32, tag="mlp", name="psmlp")
                ms = slice(128 * m, 128 * (m + 1))
                nc.tensor.matmul(ps[:], w1h[:, ms], xh[:], start=True, stop=False)
                nc.tensor.matmul(ps[:], w1h[:, ms], xl[:], start=False, stop=False)
                nc.tensor.matmul(ps[:], w1l[:, ms], xh[:], start=False, stop=True)
                th = sp.tile([128, FCH], F16, tag=f"hh{m}", name=f"hh{m}")
                nc.scalar.activation(th[:], ps[:], mybir.ActivationFunctionType.Relu)
                tl = sp.tile([128, FCH], F16, tag=f"hl{m}", name=f"hl{m}")
                nc.vector.scalar_tensor_tensor(
                    out=tl[:], in0=ps[:], scalar=0.0, in1=th[:],
                    op0=mybir.AluOpType.max, op1=mybir.AluOpType.subtract)
                hh.append(th); hl.append(tl)
                yield

            zh = []
            zl = []
            for i in range(ZBLK):
                msl = slice(128 * i, 128 * (i + 1))
                ps = pp.tile([128, FCH], F32, tag="mlp", name="psmlp")
                first = True
                for k in range(4):
                    nc.tensor.matmul(ps[:], w2h[k][:, msl], hh[k][:],
                                     start=first, stop=False); first = False
                if not b2_zero:
                    nc.tensor.matmul(ps[:], w2hb[:, msl], ones_row[:],
                                     start=False, stop=False)
                for k in range(4):
                    nc.tensor.matmul(ps[:], w2l[k][:, msl], hh[k][:],
                                     start=False, stop=False)
                if not b2_zero:
                    nc.tensor.matmul(ps[:], w2lb[:, msl], ones_row[:],
                                     start=False, stop=False)
                for k in range(4):
                    nc.tensor.matmul(ps[:], w2h[k][:, msl], hl[k][:],
                                     start=False, stop=(k == 3))
                th = sp.tile([ZR, FCH], F16, tag=f"zh{i}", name=f"zh{i}")
                nc.scalar.activation(th[:], ps[:ZR, :],
                                     mybir.ActivationFunctionType.Copy)
                tl = sp.tile([ZR, FCH], F16, tag=f"zl{i}", name=f"zl{i}")
                nc.vector.tensor_tensor(out=tl[:], in0=ps[:ZR, :], in1=th[:],
                                        op=mybir.AluOpType.subtract)
                if b2_zero:
                    # the ones row (feeds the -0.5*||c||^2 bias term) normally
                    # rides the bias matmuls; set it directly instead
                    nc.vector.memset(th[96:97, :], 1.0)
                    nc.vector.memset(tl[96:97, :], 0.0)
                nc.scalar.dma_start(d["zh_out"][128 * i:128 * i + ZR, cs], th[:])
                nc.scalar.dma_start(d["zl_out"][128 * i:128 * i + ZR, cs], tl[:])
                zh.append(th); zl.append(tl)
                yield
            chunk_z[c] = (zh, zl)

        def dec_gen(c, zqT):
            cs = slice(FCH * c, FCH * (c + 1))
            d1 = []
            for m in range(4):
                ps = pp.tile([128, FCH], F32, tag="mlp", name="psmlp")
                ms = slice(128 * m, 128 * (m + 1))
                nc.tensor.matmul(ps[:], dw1[0][:, ms], zqT[0][:], start=True,
                                 stop=False)
                nc.tensor.matmul(ps[:], dw1[1][:, ms], zqT[1][:],
                                 start=False, stop=db1_zero)
                if not db1_zero:
                    nc.tensor.matmul(ps[:], dw1b[:, ms], ones_row[:],
                                     start=False, stop=True)
                td = sp.tile([128, FCH], F16, tag=f"d1{m}", name=f"d1{m}")
                nc.scalar.activation(td[:], ps[:], mybir.ActivationFunctionType.Relu)
                d1.append(td)
                yield
            ps = pp.tile([64, FCH], F32, tag="mlp", name="psmlp2")
            for k in range(4):
                nc.tensor.matmul(ps[:], dw2[k][:], d1[k][:], start=(k == 0),
                                 stop=(db2_zero and k == 3))
            if not db2_zero:
                nc.tensor.matmul(ps[:], dw2b[:], ones_row[:], start=False, stop=True)
            bh = sp.tile([64, FCH], F32, tag="bh", name="bhT")
            nc.scalar.activation(bh[:], ps[:], mybir.ActivationFunctionType.Copy)
            nc.scalar.dma_start(d["bh_out"][:, cs], bh[:])
            yield

        import itertools

        for _ in enc_gen(0):
            pass

        prev_dec = iter(())
        for c in range(NCHUNK):
            cs = slice(FCH * c, FCH * (c + 1))
            zh, zl = chunk_z[c]
            nxt = itertools.chain(
                prev_dec, enc_gen(c + 1) if c + 1 < NCHUNK else iter(()))

            for t in range(NTILE):
                ts = slice(FT * t, FT * (t + 1))
                mxall = sp.tile([128, 8 * G], F32, tag="mxall", name="mxall",
                                bufs=3)
                idx8 = sp.tile([128, 8 * G], U32, tag="idx8", name="idx8",
                               bufs=3)
                for g in range(G):
                    i, j = divmod(g, 3)
                    dps = dpp.tile([128, K], F32, tag="dps", name="dps")
                    for kc in range(2):
                        ks = slice(512 * kc, 512 * (kc + 1))
                        cks = slice(K * j + 512 * kc, K * j + 512 * (kc + 1))
                        nc.tensor.matmul(dps[:, ks], zh[i][:, ts], cbh[i][:, cks],
                                         start=True, stop=False)
                        nc.tensor.matmul(dps[:, ks], zh[i][:, ts], cbl[i][:, cks],
                                         start=False, stop=False)
                    for kc in range(2):
                        ks = slice(512 * kc, 512 * (kc + 1))
                        cks = slice(K * j + 512 * kc, K * j + 512 * (kc + 1))
                        nc.tensor.matmul(dps[:, ks], zl[i][:, ts], cbh[i][:, cks],
                                         start=False, stop=(kc == 1))
                    # per-group argmax straight from PSUM (top-8 + first index)
                    e8 = slice(8 * g, 8 * (g + 1))
                    nc.vector.max(out=mxall[:, e8], in_=dps[:])
                    nc.vector.max_index(out=idx8[:, e8], in_max=mxall[:, e8],
                                        in_values=dps[:])
                    if g % 3 == 2:
                        next(nxt, None)
                idx = sp.tile([128, G], U32, tag="idx", name="idx", bufs=3)
                nc.vector.tensor_copy(
                    idx[:], idx8[:].rearrange("p (g e) -> p g e", g=G)[:, :, 0])
                nc.scalar.dma_start(d["idx_out"][FCH * c + FT * t:
                                                 FCH * c + FT * (t + 1), :], idx[:])

                zq = sp.tile([128, LATENT], F32, tag="zq", name="zq", bufs=3)
                for g in range(G):
                    nc.gpsimd.indirect_dma_start(
                        out=zq[:, GD * g:GD * (g + 1)], out_offset=None,
                        in_=d["cbflat"][:],
                        in_offset=bass.IndirectOffsetOnAxis(
                            ap=idx8[:, 8 * g:8 * g + 1], axis=0),
                        element_offset=g * K * GD,
                        bounds_check=K - 1, oob_is_err=False)
                nc.scalar.dma_start(d["zq_out"][FCH * c + FT * t:
                                                FCH * c + FT * (t + 1), :], zq[:])

                zq16 = sp.tile([128, LATENT], F16, tag="zq16", name="zq16",
                               bufs=3)
                nc.scalar.activation(zq16[:], zq[:], mybir.ActivationFunctionType.Copy)
                tp = pp.tile([128, LATENT], F16, tag="mlp", name="tp")
                nc.tensor.transpose(tp[:, 0:128], zq16[:, 0:128], ident[:])
                nc.tensor.transpose(tp[:, 128:256], zq16[:, 128:256], ident[:])
                if t == 0:
                    zqT = [sp.tile([128, FCH], F16, tag=f"zqT{k}", name=f"zqT{k}") for k in range(2)]
                nc.scalar.activation(zqT[0][:, ts], tp[:, 0:128],
                                     mybir.ActivationFunctionType.Copy)
                nc.scalar.activation(zqT[1][:, ts], tp[:, 128:256],
                                     mybir.ActivationFunctionType.Copy)

            for _ in nxt:
                pass
            prev_dec = dec_gen(c, zqT)

        for _ in prev_dec:
            pass

        for p in (dpp, pp, sp, wp):
            p.release()

    nc.compile()
    return nc


def _prep(inputs):
    w1 = np.asarray(inputs["enc_w1"], np.float32)
    b1 = np.asarray(inputs["enc_b1"], np.float32)
    w2 = np.asarray(inputs["enc_w2"], np.float32)
    b2 = np.asarray(inputs["enc_b2"], np.float32)
    cb = np.asarray(inputs["codebooks"], np.float32)
    dw1 = np.asarray(inputs["dec_w1"], np.float32)
    db1 = np.asarray(inputs["dec_b1"], np.float32)
    dw2 = np.asarray(inputs["dec_w2"], np.float32)
    db2 = np.asarray(inputs["dec_b2"], np.float32)

    w1a = np.vstack([w1, b1[None]])                       # (65, 512)
    w1h, w1l = _split16(w1a)

    # w2_aug: 3 blocks of 128 output columns; block i column q:
    #   q in [0,96): z dim of group 3i + q//32, q == 96: ones row
    # cb_bd: block-diagonal codebook per block, row 96 = -0.5*||c_k||^2
    w2a = np.zeros((HIDDEN + 1, ZW), np.float32)
    cbs = np.zeros((G * GD, K), np.float32)
    cbb = np.zeros((ZBLK, 3 * K), np.float32)
    for g in range(G):
        i, j = divmod(g, 3)
        w2a[:HIDDEN, 128 * i + GD * j:128 * i + GD * (j + 1)] = \
            w2[:, GD * g:GD * (g + 1)]
        w2a[HIDDEN, 128 * i + GD * j:128 * i + GD * (j + 1)] = \
            b2[GD * g:GD * (g + 1)]
        cbs[GD * g:GD * (g + 1)] = cb[g].T
        cbb[i, K * j:K * (j + 1)] = -0.5 * (cb[g] ** 2).sum(-1)
    for i in range(ZBLK):
        w2a[HIDDEN, 128 * i + 96] = 1.0
    w2h, w2l = _split16(w2a)
    cbhs, cbls = _split16(cbs)
    cbhb, cblb = _split16(cbb)

    dw1a = np.vstack([dw1, db1[None]]).astype(np.float16)  # (257, 512)
    dw2a = np.vstack([dw2, db2[None]]).astype(np.float16)  # (513, 64)

    shared = dict(w1h=w1h, w1l=w1l, w2h=w2h, w2l=w2l,
                  cbhs=cbhs, cbls=cbls, cbhb=cbhb, cblb=cblb,
                  cbflat=np.ascontiguousarray(cb.reshape(G * K, GD)),
                  dw1=dw1a, dw2=dw2a)

    bands = np.asarray(inputs["bands"], np.float32)
    in_maps = []
    for cix in range(N_CORES):
        xc = bands[2 * cix:2 * cix + 2].reshape(NC_FRAMES, NUM_BANDS)
        xT = np.zeros((65, NPAD), np.float32)
        xT[:NUM_BANDS, :NC_FRAMES] = xc.T
        xT[NUM_BANDS, :] = 1.0
        xh, xl = _split16(xT)
        in_maps.append(dict(shared, xh=xh, xl=xl))
    return in_maps


def kernel(**inputs):
    global LAST_EXEC_NS
    flags = (not np.any(np.asarray(inputs["enc_b2"])),
             not np.any(np.asarray(inputs["dec_b1"])),
             not np.any(np.asarray(inputs["dec_b2"])))
    if flags not in _CACHE:
        _CACHE[flags] = _build(*flags)
    nc = _CACHE[flags]

    in_maps = _prep(inputs)
    trace = bool(int(os.environ.get("KERNEL_TRACE", "0")))
    kw = {}
    pdir = os.environ.get("KERNEL_PROF_DIR")
    if pdir:
        os.makedirs(pdir, exist_ok=True)
        kw["tmpdir"] = pdir
    res = run_bass_kernel_spmd(nc, in_maps, core_ids=list(range(N_CORES)),
                               trace=trace, **kw)
    LAST_EXEC_NS = res.exec_time_ns

    z_e = np.empty((N_CORES, NC_FRAMES, LATENT), np.float32)
    z_q = np.empty((N_CORES, NC_FRAMES, LATENT), np.float32)
    idx = np.empty((N_CORES, NC_FRAMES, G), np.int32)
    bh = np.empty((N_CORES, NC_FRAMES, NUM_BANDS), np.float32)
    for cix in range(N_CORES):
        r = res.results[cix]
        zT = r["zh_out"].astype(np.float32) + r["zl_out"].astype(np.float32)
        for g in range(G):
            i, j = divmod(g, 3)
            base = 128 * i + GD * j
            z_e[cix, :, GD * g:GD * (g + 1)] = zT[base:base + GD, :NC_FRAMES].T
        z_q[cix] = r["zq_out"][:NC_FRAMES]
        idx[cix] = r["idx_out"][:NC_FRAMES].astype(np.int32)
        bh[cix] = r["bh_out"][:, :NC_FRAMES].T

    z_e = z_e.reshape(B, T, LATENT)
    z_q = z_q.reshape(B, T, LATENT)
    idx = idx.reshape(B, T, G)
    bands_hat = bh.reshape(B, T, NUM_BANDS)

    z_q_st = z_e + (z_q - z_e)

    dif = (z_q - z_e).reshape(B * T, G, GD)
    m = np.mean(dif * dif, axis=(0, 2), dtype=np.float32)
    cl = m.sum(dtype=np.float32)
    vq_loss = np.float32(BETA) * (cl + cl)

    return (bands_hat, z_e, z_q_st, idx, np.float32(vq_loss))
